# revision 1
# baseline (speedup 1.0000x reference)
"""AlignmentModule (conv stacks -> L2 distance -> log_softmax + beta-binomial
prior) on 8 Trainium2 NeuronCores, data-parallel over batch (2 per core).

All matmuls run as fp8e4 DoubleRow (contraction 256/instr at 0.5 cyc/row, 4x
the fp32r rate): weights and activations are quantized to e4m3 on host with
power-of-2 scales folded into eviction constants; biases enter each PSUM via
rank-1 DoubleRow bias-matmuls (the f1 bias rides a ones-row shipped inside
featsT), so every PSUM eviction is a pure (mult s)[(max 0)] TensorScalar.
||t||^2 / ||f||^2 row-sums are DoubleRow ones-matmuls over fp8 squares and
join the score PSUM through a 1-partition augmented DoubleRow matmul (8*x/8
pairs keep e4m3 exact). The softmax is shift-free -- dist >= 0 bounds
exp(K - dist) via a +25 constant folded into the fp16 prior -- with Ln batched
per 2-tile group, and the final prior - cc - dist combine is one all-SBUF STT
(or a Pool TT + DVE TSP split for every other tile, offloading the 0.594us
no-fast-mode STT).

Engine budget per core (cost model): ACT ~51us (sqrt pairs at free-1024, exp
+accum, f1 relu evictions), DVE ~53us (conv evictions, row evictions, final
combines), Pool ~27us (fp8 squares, pads), PE ~35us, DMA ~32us. Emission is
software-pipelined: a diagonal conv wavefront over 3 2-bank PSUM slots,
scores lagging their chunk by one, per-group softmax tails, and batch 1's
stream staggered 30 steps behind batch 0 so its head fills batch 0's tail.

Pool engine cannot touch PSUM, STT allows at most one PSUM operand, and
DoubleRow ldweights needs stationary M >= 32 -- all discovered via walrus
probes; see the git-less history in this file's evolution.
"""

import numpy as np

import concourse.bass as bass
import concourse.mybir as mybir
from concourse.tile import TileContext

F32 = mybir.dt.float32
F16 = mybir.dt.float16
BF16 = mybir.dt.bfloat16
F8 = mybir.dt.float8e4
AF = mybir.ActivationFunctionType
OP = mybir.AluOpType
DR = mybir.MatmulPerfMode.DoubleRow

B, T_TEXT, T_FEATS, ADIM, ODIM = 16, 512, 2048, 256, 80
N_CORES = 8
B_LOC = B // N_CORES
NT = T_FEATS // 512   # 4 f time chunks
NF = T_FEATS // 128   # 16 score tiles per batch

# host-side power-of-2 quantization scales per layer (weights only; acts at 1)
S_T1, S_T2, S_F1, S_F2, S_F3 = 32.0, 8.0, 16.0, 32.0, 16.0
K_SHIFT = 25.0  # exp(-dist + K); prior has +K folded in on host


def _split_excess_waits(nc, limit=1):
    """walrus CoreV3 CTRL codegen rejects >1 sync-wait per instruction.
    Hoist excess waits onto preceding NOPs on the same engine."""
    ctr = 0
    for f in nc.m.functions:
        for bb in f.blocks:
            insts = bb.instructions
            idx = 0
            while idx < len(insts):
                ins = insts[idx]
                si = ins.sync_info
                if si is not None and len(si.on_wait) > limit:
                    waits = list(si.on_wait)
                    extra, keep = waits[:-limit], waits[-limit:]
                    si.on_wait = keep
                    pos = idx
                    for j in range(0, len(extra), limit):
                        nop = mybir.InstNoOp(name=f"waitsplit_{ctr}", ins=[], outs=[])
                        ctr += 1
                        nop.engine = ins.engine
                        nop.sync_info = mybir.SyncInfo(
                            on_wait=extra[j : j + limit], on_update=[]
                        )
                        insts.insert(pos, nop)
                        pos += 1
                        idx += 1
                idx += 1
    return ctr


def _beta_binomial_prior():
    from scipy.special import gammaln

    T, N = T_FEATS, T_TEXT
    a = np.arange(1, T + 1, dtype=np.float64)[:, None]
    b = (T - np.arange(1, T + 1, dtype=np.float64) + 1.0)[:, None]
    k = np.arange(N, dtype=np.float64)[None, :]
    n = float(N)

    def betaln(x, y):
        return gammaln(x) + gammaln(y) - gammaln(x + y)

    logp = (
        gammaln(n + 1.0)
        - gammaln(k + 1.0)
        - gammaln(n - k + 1.0)
        + betaln(k + a, n - k + b)
        - betaln(a, b)
    )
    return logp.astype(np.float32)


def _build_nc(zero_bias=False):
    nc = bass.Bass(name="alignment")
    TT, TF = T_TEXT, T_FEATS

    # --- DRAM I/O (per core); all activations/weights pre-quantized e4m3 ---
    textT = nc.dram_tensor("textT", [B_LOC, 128, 2, TT], F8, kind="ExternalInput")
    featsT = nc.dram_tensor("featsT", [B_LOC, ODIM + 1, TF], F8,
                            kind="ExternalInput")
    tw1 = nc.dram_tensor("tw1", [128, 3, 2, 2, 128], F8, kind="ExternalInput")
    tw2 = nc.dram_tensor("tw2", [128, 2, 2, 128], F8, kind="ExternalInput")
    fw1 = nc.dram_tensor("fw1", [ODIM + 1, 3, 2, 128], F8, kind="ExternalInput")
    fw2 = nc.dram_tensor("fw2", [128, 3, 2, 2, 128], F8, kind="ExternalInput")
    fw3 = nc.dram_tensor("fw3", [128, 2, 2, 128], F8, kind="ExternalInput")
    # rank-1 bias rows for t1, t2, f2, f3: [layer, m, mcol] values b*s
    biasA = nc.dram_tensor("biasA", [1, 4, 2, 128], F8, kind="ExternalInput")
    priorD = nc.dram_tensor("prior", [128, NF, TT], F16, kind="ExternalInput")
    outD = nc.dram_tensor("out", [B_LOC, TF, TT], F32, kind="ExternalOutput")

    with TileContext(nc) as tc:
        with (
            tc.tile_pool(name="const", bufs=1) as const,
            tc.tile_pool(name="wpool", bufs=1) as wpool,
            tc.tile_pool(name="tx0p", bufs=2) as tx0p,
            tc.tile_pool(name="txp", bufs=2) as txp,
            tc.tile_pool(name="ft0p", bufs=2) as ft0p,
            tc.tile_pool(name="ftp", bufs=2) as ftp,
            tc.tile_pool(name="augp", bufs=2) as augp,
            tc.tile_pool(name="distp", bufs=2) as distp,
            tc.tile_pool(name="ep", bufs=2) as ep,
            tc.tile_pool(name="pdp", bufs=4) as pdp,
            tc.tile_pool(name="ssp", bufs=2) as ssp,
            tc.tile_pool(name="outp", bufs=6) as outpool,
            tc.tile_pool(name="pp", bufs=3, space="PSUM") as pp,
            tc.tile_pool(name="pps", bufs=1, space="PSUM") as pps,
        ):
            # ---- weights / constants (one-time) ----
            # ---- constants (memsets early, on idle engines) ----
            ones2col = const.tile([128, 2, 32], F8, tag="ones2col")
            nc.vector.memset(ones2col[:], 1.0)
            ones2row = const.tile([1, 2, TT], F8, tag="ones2row")
            nc.vector.memset(ones2row[0:1, 0, :], 1.0)
            nc.vector.memset(ones2row[0:1, 1, :], 0.0)
            eps1 = const.tile([128, 1], F32, tag="eps1")
            nc.vector.memset(eps1[:], 1.0)
            kshift = const.tile([128, 1], F32, tag="kshift")
            nc.vector.memset(kshift[:], K_SHIFT)

            tw1_sb = wpool.tile([128, 3, 2, 2, 128], F8, tag="tw1")
            tw2_sb = wpool.tile([128, 2, 2, 128], F8, tag="tw2")
            fw1_sb = wpool.tile([ODIM + 1, 3, 2, 128], F8, tag="fw1")
            fw2_sb = wpool.tile([128, 3, 2, 2, 128], F8, tag="fw2")
            fw3_sb = wpool.tile([128, 2, 2, 128], F8, tag="fw3")
            biasA_sb = wpool.tile([1, 4, 2, 128], F8, tag="biasA")
            prior_sb = const.tile([128, NF, TT], F16, tag="prior")

            def emit_tweights():
                nc.sync.dma_start(out=tw1_sb[:], in_=tw1[:])
                nc.sync.dma_start(out=tw2_sb[:], in_=tw2[:])
                if not zero_bias:
                    nc.sync.dma_start(out=biasA_sb[:], in_=biasA[:])

            def emit_fweights():
                nc.sync.dma_start(out=fw1_sb[:], in_=fw1[:])
                nc.sync.dma_start(out=fw2_sb[:], in_=fw2[:])
                nc.sync.dma_start(out=fw3_sb[:], in_=fw3[:])

            def emit_prior():
                nc.sync.dma_start(out=prior_sb[:], in_=priorD[:])

            states = [{} for _ in range(B_LOC)]

            def emit_loads_tx(b, st):
                # text activations [128, 2, TT+2] with zero pads at cols 0, TT+1
                tx0 = tx0p.tile([128, 2, TT + 2], F8, tag="tx0")
                nc.vector.memset(tx0[:, :, 0:1], 0.0)
                nc.vector.memset(tx0[:, :, TT + 1 : TT + 2], 0.0)
                nc.gpsimd.dma_start(out=tx0[:, :, 1 : TT + 1], in_=textT[b])
                tx1 = txp.tile([128, 2, TT + 2], F8, tag="tx1")
                nc.vector.memset(tx1[:, :, 0:1], 0.0)
                nc.vector.memset(tx1[:, :, TT + 1 : TT + 2], 0.0)
                st.update(tx0=tx0, tx1=tx1)

            def emit_loads_ft(b, st):
                # ft0: row 0 = ones/bias row (shipped inside featsT)
                ft0 = ft0p.tile([ODIM + 1, TF + 2], F8, tag="ft0")
                nc.gpsimd.memset(ft0[:, 0:1], 0.0)
                nc.gpsimd.memset(ft0[:, TF + 1 : TF + 2], 0.0)
                nc.gpsimd.dma_start(out=ft0[:, 1 : TF + 1], in_=featsT[b])
                ft1 = ftp.tile([128, 2, TF + 2], F8, tag="ft1")
                nc.gpsimd.memset(ft1[:, :, 0:1], 0.0)
                nc.gpsimd.memset(ft1[:, :, TF + 1 : TF + 2], 0.0)
                ft2 = ftp.tile([128, 2, TF + 2], F8, tag="ft2")
                nc.gpsimd.memset(ft2[:, :, 0:1], 0.0)
                nc.gpsimd.memset(ft2[:, :, TF + 1 : TF + 2], 0.0)
                ft3 = ftp.tile([128, 2, TF], F8, tag="ft3")
                ftsq = ftp.tile([128, 2, TF], F8, tag="ftsq")
                f2aug = augp.tile([1, 2, TF], F8, tag="f2aug")
                t2aug = augp.tile([1, 2, TT], F8, tag="t2aug")
                dist = distp.tile([128, NF, TT], F16, tag="dist")
                ssum = ssp.tile([128, NF], F32, tag="ssum")
                lns = ssp.tile([128, NF], F32, tag="lns")
                st.update(ft0=ft0, ft1=ft1, ft2=ft2,
                          ft3=ft3, ftsq=ftsq, f2aug=f2aug, t2aug=t2aug,
                          dist=dist, ssum=ssum, lns=lns)

            def emit_aug_consts(b, st):
                nc.gpsimd.memset(st["f2aug"][0:1, 0, :], 8.0)
                nc.gpsimd.memset(st["t2aug"][0:1, 1, :], 8.0)

            def emit_T1(b, st):
                tx0, tx1 = st["tx0"], st["tx1"]
                # t1: PSUM = S_T1*(W1 x + b1); 3 DR (k) per m + bias DR
                T1 = pp.tile([128, 2, TT], F32, tag="ppc")
                for m in range(2):
                    for k in range(3):
                        nc.tensor.matmul(
                            T1[:, m, :],
                            tw1_sb[:, k, :, m, :],
                            tx0[:, :, k : k + TT],
                            start=(k == 0),
                            stop=(zero_bias and k == 2),
                            perf_mode=DR,
                        )
                    if not zero_bias:
                        nc.tensor.matmul(
                            T1[:, m, :], biasA_sb[:, 0], ones2row[:],
                            start=False, stop=True, perf_mode=DR,
                        )
                # evict: tx1 = relu(T1)/S_T1 (DVE, merged free-1024)
                nc.vector.tensor_scalar(
                    tx1[:, :, 1 : TT + 1], T1[:], 1.0 / S_T1, 0.0, OP.mult, OP.max
                )

            def emit_T2(b, st):
                tx1 = st["tx1"]
                # t2: PSUM = S_T2*(-2)*(W2 tx1 + b2)
                T2 = pp.tile([128, 2, TT], F32, tag="ppc")
                for m in range(2):
                    nc.tensor.matmul(
                        T2[:, m, :], tw2_sb[:, :, m, :], tx1[:, :, 1 : TT + 1],
                        start=True, stop=zero_bias, perf_mode=DR,
                    )
                    if not zero_bias:
                        nc.tensor.matmul(
                            T2[:, m, :], biasA_sb[:, 1], ones2row[:],
                            start=False, stop=True, perf_mode=DR,
                        )
                tx2m2 = txp.tile([128, 2, TT], F8, tag="tx2m2")
                nc.vector.tensor_scalar(
                    tx2m2[:], T2[:], 1.0 / S_T2, None, OP.mult
                )
                # txsq = t^2 = (T2/32) * (-2t)... (T2 = -16t): one PSUM +
                # one SBUF operand is legal on DVE; keeps Pool off the
                # first-score critical path
                txsq = txp.tile([128, 2, TT], F8, tag="txsq")
                nc.vector.scalar_tensor_tensor(
                    txsq[:], T2[:], 1.0 / 32.0, tx2m2[:], OP.mult, OP.mult
                )
                st["tx2m2"] = tx2m2
                st["txsq"] = txsq

            def emit_t2row_mm(b, st):
                # t2row: psn = sum_c 4 t_c^2 = 4*t2 (PE); evict deferred
                psnt = pps.tile([128, 2, TT], F32, tag="pscore")
                nc.tensor.matmul(psnt[0:32, 0, :], ones2col[:], st["txsq"],
                                 start=True, stop=True, perf_mode=DR)
                st["t2psn"] = psnt

            def emit_t2row_evict(b, st):
                nc.vector.tensor_scalar(
                    st["t2aug"][0:1, 0, :], st["t2psn"][0:1, 0, :],
                    1.0 / 32.0, None, OP.mult
                )

            def emit_f1(b, st, n):
                ft0, ft1 = st["ft0"], st["ft1"]
                o = n * 512
                # f1: contraction 81 (80 ch + ones/bias row), 3 taps, plain fp8
                F1 = pp.tile([128, 2, 512], F32, tag="ppc")
                for m in range(2):
                    for k in range(3):
                        nc.tensor.matmul(
                            F1[:, m, :],
                            fw1_sb[:, k, m, :],
                            ft0[:, o + k : o + k + 512],
                            start=(k == 0),
                            stop=(k == 2),
                        )
                if n != 3:
                    nc.scalar.activation(
                        ft1[:, :, 1 + o : 1 + o + 512], F1[:], AF.Relu,
                        scale=1.0 / S_F1,
                    )
                else:
                    nc.vector.tensor_scalar(
                        ft1[:, :, 1 + o : 1 + o + 512], F1[:], 1.0 / S_F1,
                        0.0, OP.mult, OP.max,
                    )

            def emit_f2(b, st, n):
                ft1, ft2 = st["ft1"], st["ft2"]
                o = n * 512
                # f2: 3 DR + bias DR per m
                F2 = pp.tile([128, 2, 512], F32, tag="ppc")
                for m in range(2):
                    for k in range(3):
                        nc.tensor.matmul(
                            F2[:, m, :],
                            fw2_sb[:, k, :, m, :],
                            ft1[:, :, o + k : o + k + 512],
                            start=(k == 0),
                            stop=(zero_bias and k == 2),
                            perf_mode=DR,
                        )
                    if not zero_bias:
                        nc.tensor.matmul(
                            F2[:, m, :], biasA_sb[:, 2], ones2row[:, :, 0:512],
                            start=False, stop=True, perf_mode=DR,
                        )
                nc.vector.tensor_scalar(
                    ft2[:, :, 1 + o : 1 + o + 512], F2[:], 1.0 / S_F2, 0.0,
                    OP.mult, OP.max,
                )

            def emit_f3(b, st, n):
                ft2, ft3, ftsq = st["ft2"], st["ft3"], st["ftsq"]
                o = n * 512
                # f3: 1 DR + bias DR per m
                F3 = pp.tile([128, 2, 512], F32, tag="ppc")
                for m in range(2):
                    nc.tensor.matmul(
                        F3[:, m, :], fw3_sb[:, :, m, :],
                        ft2[:, :, 1 + o : 1 + o + 512],
                        start=True, stop=zero_bias, perf_mode=DR,
                    )
                    if not zero_bias:
                        nc.tensor.matmul(
                            F3[:, m, :], biasA_sb[:, 3], ones2row[:, :, 0:512],
                            start=False, stop=True, perf_mode=DR,
                        )
                sl = slice(o, o + 512)
                nc.vector.tensor_scalar(
                    ft3[:, :, sl], F3[:], 1.0 / S_F3, None, OP.mult
                )
                nc.gpsimd.tensor_tensor(
                    ftsq[:, :, sl], ft3[:, :, sl], ft3[:, :, sl], OP.mult
                )

            def emit_f2row_mm(b, st, n):
                sl = slice(n * 512, (n + 1) * 512)
                psnt = pps.tile([128, 2, 512], F32, tag="pscore")
                nc.tensor.matmul(psnt[0:32, 0, :], ones2col[:],
                                 st["ftsq"][:, :, sl],
                                 start=True, stop=True, perf_mode=DR)
                st["fpsn%d" % n] = psnt

            def emit_f2row_evict(b, st, n):
                sl = slice(n * 512, (n + 1) * 512)
                nc.vector.tensor_scalar(
                    st["f2aug"][0:1, 1, sl], st["fpsn%d" % n][0:1, 0, :],
                    0.125, None, OP.mult
                )

            def emit_score_pair(b, st, i):
                # score tiles i, i+1 into one 2-bank PSUM; sqrt at free-1024
                tx2m2, t2aug = st["tx2m2"], st["t2aug"]
                ft3, f2aug = st["ft3"], st["f2aug"]
                dist, ssum = st["dist"], st["ssum"]
                S2 = pps.tile([128, 2, TT], F32, tag="pscore")
                for j in range(2):
                    fsl = slice((i + j) * 128, (i + j + 1) * 128)
                    nc.tensor.matmul(S2[:, j, :], ft3[:, :, fsl], tx2m2[:],
                                     start=True, stop=False, perf_mode=DR)
                    nc.tensor.matmul(S2[:, j, :], f2aug[:, :, fsl], t2aug[:],
                                     start=False, stop=True, perf_mode=DR)
                nc.scalar.activation(dist[:, i : i + 2, :], S2[:], AF.Sqrt,
                                     bias=eps1[:])
                for j in range(2):
                    e = ep.tile([128, TT], BF16, tag="e")
                    nc.scalar.activation(
                        e[:], dist[:, i + j, :], AF.Exp, scale=-1.0,
                        bias=kshift[:], accum_out=ssum[:, i + j : i + j + 1],
                    )

            def emit_tail_group(b, st, g):
                # g indexes pairs of score tiles (8 groups of 2 per batch)
                dist, ssum, lns = st["dist"], st["ssum"], st["lns"]
                sl2 = slice(2 * g, 2 * g + 2)
                nc.scalar.activation(lns[:, sl2], ssum[:, sl2], AF.Ln)
                o2 = outpool.tile([128, 2, TT], F32, tag="o4")
                # tile j=0 (and j=1 on odd groups) via Pool TT + DVE TSP,
                # offloading the no-fast-mode STT from DVE
                i0 = g * 2
                pd = pdp.tile([128, TT], F16, tag="pd")
                nc.gpsimd.tensor_tensor(
                    pd[:], prior_sb[:, i0, :], dist[:, i0, :], OP.subtract
                )
                nc.vector.tensor_scalar(
                    o2[:, 0, :], pd[:], lns[:, i0 : i0 + 1], None, OP.subtract
                )
                if g % 2 == 1:
                    pd2 = pdp.tile([128, TT], F16, tag="pd")
                    nc.gpsimd.tensor_tensor(
                        pd2[:], prior_sb[:, i0 + 1, :], dist[:, i0 + 1, :],
                        OP.subtract
                    )
                    nc.vector.tensor_scalar(
                        o2[:, 1, :], pd2[:], lns[:, i0 + 1 : i0 + 2], None,
                        OP.subtract
                    )
                else:
                    nc.vector.scalar_tensor_tensor(
                        o2[:, 1, :], prior_sb[:, i0 + 1, :],
                        lns[:, i0 + 1 : i0 + 2],
                        dist[:, i0 + 1, :], OP.subtract, OP.subtract,
                    )
                if g >= 3:
                    # terminal groups: per-tile DMAs so the last transfer
                    # only waits on its own tile
                    for j in range(2):
                        nc.sync.dma_start(
                            out=outD[b, (2 * g + j) * 128 : (2 * g + j + 1) * 128,
                                     :].rearrange("(i p) t -> p i t", p=128),
                            in_=o2[:, j : j + 1, :],
                        )
                else:
                    nc.sync.dma_start(
                        out=outD[b, g * 256 : (g + 1) * 256, :].rearrange(
                            "(i p) t -> p i t", p=128
                        ),
                        in_=o2[:],
                    )

            # ---------- software-pipelined schedule ----------
            # per batch: scores lag their conv chunk by one so the square ->
            # row-sum -> aug-evict chain never blocks the next chunk's PSUM
            # slot rotation or the DVE queue.
            def emit_all(b, st):
                emit_aug_consts(b, st)
                emit_T1(b, st)
                yield
                emit_f1(b, st, 0)
                yield
                emit_f1(b, st, 1)
                yield
                emit_T2(b, st)
                yield
                emit_f2(b, st, 0)
                yield
                emit_f1(b, st, 2)
                yield
                emit_t2row_mm(b, st)
                yield
                emit_t2row_evict(b, st)
                yield
                emit_f2(b, st, 1)
                yield
                emit_f3(b, st, 0)
                yield
                emit_f2row_mm(b, st, 0)
                yield
                emit_f2row_evict(b, st, 0)
                yield
                emit_f1(b, st, 3)
                yield
                emit_f2(b, st, 2)
                yield
                emit_f3(b, st, 1)
                yield
                emit_score_pair(b, st, 0)
                yield
                emit_score_pair(b, st, 2)
                yield
                emit_f2(b, st, 3)
                yield
                emit_f2row_mm(b, st, 1)
                yield
                emit_f3(b, st, 2)
                yield
                emit_f2row_evict(b, st, 1)
                yield
                emit_score_pair(b, st, 4)
                yield
                emit_tail_group(b, st, 0)
                yield
                emit_score_pair(b, st, 6)
                yield
                emit_tail_group(b, st, 1)
                yield
                emit_f3(b, st, 3)
                yield
                emit_f2row_mm(b, st, 2)
                yield
                emit_f2row_evict(b, st, 2)
                yield
                emit_score_pair(b, st, 8)
                yield
                emit_tail_group(b, st, 2)
                yield
                emit_score_pair(b, st, 10)
                yield
                emit_tail_group(b, st, 3)
                yield
                emit_tail_group(b, st, 4)
                yield
                emit_f2row_mm(b, st, 3)
                yield
                emit_f2row_evict(b, st, 3)
                yield
                emit_tail_group(b, st, 5)
                yield
                emit_score_pair(b, st, 12)
                yield
                emit_tail_group(b, st, 6)
                yield
                emit_score_pair(b, st, 14)
                yield
                emit_tail_group(b, st, 7)
                yield

            emit_loads_tx(0, states[0])
            emit_loads_ft(0, states[0])
            emit_tweights()
            emit_fweights()
            emit_loads_tx(1, states[1])
            emit_loads_ft(1, states[1])
            emit_prior()
            import os as _os
            STAGGER = int(_os.environ.get("KN_STAGGER", "30"))
            g0 = emit_all(0, states[0])
            g1 = emit_all(1, states[1])
            for _ in range(STAGGER):
                next(g0, None)
            alive = [g0, g1]
            while alive:
                for g in list(alive):
                    if next(g, "END") == "END":
                        alive.remove(g)

    _split_excess_waits(nc)
    return nc


def _prep_inputs(text, feats, t_w1, t_b1, t_w2, t_b2,
                 f_w1, f_b1, f_w2, f_b2, f_w3, f_b3):
    import ml_dtypes

    E4 = ml_dtypes.float8_e4m3
    c = np.ascontiguousarray
    f4 = np.float32

    def q(x):
        return c(x).astype(E4)

    # text: [B, T, C] -> [B, 128, 2, T] : [b, p, cpair, t] = text[b, t, cpair*128+p]
    tx = text.astype(f4).transpose(0, 2, 1).reshape(B, 2, 128, T_TEXT)
    tx = tx.transpose(0, 2, 1, 3)  # [B, 128, 2, T]
    # feats: [B, T, 80] -> [B, 81, T] with ones row 0 (f1 bias row)
    ftT = feats.astype(f4).transpose(0, 2, 1)
    ft = np.ones((B, ODIM + 1, T_FEATS), f4)
    ft[:, 1:] = ftT

    def conv_w_dr(w, s):
        # torch [C_out, C_in, K] -> [p, k, c, m, mcol] = w[m*128+mcol, c*128+p, k]*s
        co, ci, K = w.shape
        v = (w.astype(f4) * s).transpose(2, 1, 0)  # [K, C_in, C_out]
        v = v.reshape(K, 2, 128, 2, 128)  # [k, c, p, m, mcol]
        return q(v.transpose(2, 0, 1, 3, 4))  # [p, k, c, m, mcol]

    def conv_w1_dr(w, s):
        # K=1: [p, c, m, mcol]
        v = (w.astype(f4)[:, :, 0] * s).T  # [C_in, C_out]
        v = v.reshape(2, 128, 2, 128)  # [c, p, m, mcol]
        return q(v.transpose(1, 0, 2, 3))

    # f1: [256, 80, 3] -> [81, k, m, mcol]; row 0 at k=1 = f_b1*s
    v = (f_w1.astype(f4) * S_F1).transpose(2, 1, 0)  # [3, 80, 256]
    fw1 = np.zeros((ODIM + 1, 3, 2, 128), f4)
    fw1[1:] = v.transpose(1, 0, 2).reshape(ODIM, 3, 2, 128)
    fw1[0, 1] = (f_b1.astype(f4) * S_F1).reshape(2, 128)

    biasA = np.zeros((1, 4, 2, 128), f4)
    biasA[0, 0] = (t_b1.astype(f4) * S_T1).reshape(2, 128)
    biasA[0, 1] = (t_b2.astype(f4) * (-2.0 * S_T2)).reshape(2, 128)
    biasA[0, 2] = (f_b2.astype(f4) * S_F2).reshape(2, 128)
    biasA[0, 3] = (f_b3.astype(f4) * S_F3).reshape(2, 128)

    prior = _beta_binomial_prior() + K_SHIFT  # [TF, TT]
    prior_r = prior.reshape(NF, 128, T_TEXT).transpose(1, 0, 2)  # [p, i, t]

    shared = {
        "tw1": conv_w_dr(t_w1, S_T1),
        "tw2": conv_w1_dr(t_w2, -2.0 * S_T2),
        "fw1": q(fw1),
        "fw2": conv_w_dr(f_w2, S_F2),
        "fw3": conv_w1_dr(f_w3, S_F3),
        "biasA": q(biasA),
        "prior": c(prior_r).astype(np.float16),
    }
    in_maps = []
    for core in range(N_CORES):
        m = dict(shared)
        m["textT"] = q(tx[core * B_LOC : (core + 1) * B_LOC])
        m["featsT"] = q(ft[core * B_LOC : (core + 1) * B_LOC])
        in_maps.append(m)
    return in_maps


_NC = {}
_NC_LAST = False


def _get_nc(zero_bias=None):
    global _NC_LAST
    if zero_bias is None:
        zero_bias = _NC_LAST
    _NC_LAST = zero_bias
    if zero_bias not in _NC:
        _NC[zero_bias] = _build_nc(zero_bias)
    return _NC[zero_bias]


_CALLABLE = {}


def _build_callable():
    """Compile once; return a function(in_maps) -> list of per-core output
    dicts, keeping the jitted executable alive across kernel() calls."""
    import jax
    import jax.numpy as jnp
    from jax.sharding import Mesh, NamedSharding, PartitionSpec
    from jax.experimental.shard_map import shard_map
    from concourse.bass2jax import (
        _bass_exec_p,
        install_neuronx_cc_hook,
        partition_id_tensor,
    )

    nc = _get_nc()
    install_neuronx_cc_hook()
    partition_name = nc.partition_id_tensor.name if nc.partition_id_tensor else None
    in_names, out_names, out_avals, zero_shapes = [], [], [], []
    for alloc in nc.m.functions[0].allocations:
        if not isinstance(alloc, mybir.MemoryLocationSet):
            continue
        name = alloc.memorylocations[0].name
        if alloc.kind == "ExternalInput":
            if name != partition_name:
                in_names.append(name)
        elif alloc.kind == "ExternalOutput":
            shape = tuple(alloc.tensor_shape)
            dtype = mybir.dt.np(alloc.dtype)
            out_names.append(name)
            out_avals.append(jax.core.ShapedArray(shape, dtype))
            zero_shapes.append(((N_CORES * shape[0],) + shape[1:], dtype))
    n_params = len(in_names)
    n_outs = len(out_avals)
    all_in_names = list(in_names) + out_names
    if partition_name is not None:
        all_in_names.append(partition_name)
    donate = tuple(range(n_params, n_params + n_outs))

    def _body(*args):
        operands = list(args)
        if partition_name is not None:
            operands.append(partition_id_tensor())
        outs = _bass_exec_p.bind(
            *operands,
            out_avals=tuple(out_avals),
            in_names=tuple(all_in_names),
            out_names=tuple(out_names),
            lowering_input_output_aliases=(),
            sim_require_finite=True,
            sim_require_nnan=True,
            nc=nc,
        )
        return tuple(outs)

    devices = jax.devices()[:N_CORES]
    mesh = Mesh(np.asarray(devices), ("core",))
    fn = jax.jit(
        shard_map(
            _body,
            mesh=mesh,
            in_specs=(PartitionSpec("core"),) * (n_params + n_outs),
            out_specs=(PartitionSpec("core"),) * n_outs,
            check_rep=False,
        ),
        donate_argnums=donate,
        keep_unused=True,
    )
    sharding = NamedSharding(mesh, PartitionSpec("core"))
    zfn = jax.jit(
        lambda: tuple(jnp.zeros(s, d) for s, d in zero_shapes),
        out_shardings=tuple(sharding for _ in zero_shapes),
    )

    def call(in_maps):
        concat_in = [
            np.concatenate([np.asarray(in_maps[c][n]) for c in range(N_CORES)], axis=0)
            for n in in_names
        ]
        out_arrs = fn(*concat_in, *zfn())
        return [
            {
                name: np.asarray(out_arrs[i]).reshape(
                    N_CORES, *out_avals[i].shape
                )[c]
                for i, name in enumerate(out_names)
            }
            for c in range(N_CORES)
        ]

    return call


def _run(inputs, **kw):
    global _CALLABLE
    import time as _time

    in_maps = _prep_inputs(
        inputs["text"], inputs["feats"],
        inputs["t_w1"], inputs["t_b1"], inputs["t_w2"], inputs["t_b2"],
        inputs["f_w1"], inputs["f_b1"], inputs["f_w2"], inputs["f_b2"],
        inputs["f_w3"], inputs["f_b3"],
    )
    zb = all(
        not np.any(np.asarray(inputs[k]))
        for k in ("t_b1", "t_b2", "f_b1", "f_b2", "f_b3")
    )
    _get_nc(zb)  # select variant for this call (and for later sims)
    results = None
    last_err = None
    if _CALLABLE.get(zb) is not False:
        for attempt in range(3):
            try:
                if zb not in _CALLABLE:
                    from concourse._compat import axon_active

                    if not axon_active():
                        raise RuntimeError("axon not active; use native path")
                    _CALLABLE[zb] = _build_callable()
                results = _CALLABLE[zb](in_maps)
                break
            except Exception as e:
                last_err = e
                results = None
                if attempt < 2:
                    _time.sleep(20 * (attempt + 1))
        if results is None:
            _CALLABLE[zb] = False
    if results is None:
        from concourse.bass_utils import run_bass_kernel_spmd

        for attempt in range(3):
            try:
                results = run_bass_kernel_spmd(
                    _get_nc(), in_maps, core_ids=list(range(N_CORES))
                ).results
                break
            except Exception as e:
                last_err = e
                results = None
                if attempt < 2:
                    _time.sleep(20 * (attempt + 1))
    if results is None:
        raise last_err
    out = np.concatenate([r["out"] for r in results], axis=0)
    return out, results


def kernel(**inputs) -> np.ndarray:
    out, _ = _run(inputs)
    return out

